# revision 7
# baseline (speedup 1.0000x reference)
"""AblationRouter Trainium2 kernel: GRU-routed MoE scoring, data-parallel over batch
across 8 NeuronCores.

Per core (batch row): xW/expression projections (fp32 PE matmuls), a 1024-step GRU
scan whose recurrent matmul runs as 3 split-precision passes (W2hi@h_hi + W2hi@h_lo
in float32r + W2lo@h_bf in bfloat16 ~= fp32 accuracy at 3 cycles/row), gates computed
in a transposed [128,8] layout, then per-token cosine scores + top-2 + softmax.
"""
import sys
sys.path.insert(0, "/opt/trn_rl_repo")
import numpy as np
from contextlib import ExitStack

import concourse.bass as bass
import concourse.mybir as mybir
import concourse.tile as tile
from concourse import bacc
from concourse.bass import ds, ts
from concourse.bass_utils import run_bass_kernel_spmd
from concourse.masks import make_identity

F32 = mybir.dt.float32
F32R = mybir.dt.float32r
BF16 = mybir.dt.bfloat16
I32 = mybir.dt.int32
AF = mybir.ActivationFunctionType
ALU = mybir.AluOpType
AX = mybir.AxisListType

S = 1024        # sequence length (local tokens per core)
HID = 1024      # input dim
H = 1024        # hidden dim
H3 = 3072
E = 8
D = 128
NCORES = 8

_NC_CACHE = {}


def _transpose_to(nc, tc, pools, dram_src, dst_tiles, rows, cols, ident):
    """dst[c][:, r_tile*128 ...] = dram_src[rows, cols].T, via PE transpose.
    dram_src: [rows, cols]; dst_tiles: list of col-chunk tiles [128, rows]."""
    ld, psum, cpy = pools
    for cc in range(cols // 128):
        for rc in range(rows // 128):
            t = ld.tile([128, 128], F32, tag="tr_ld")
            nc.sync.dma_start(t[:], dram_src[ts(rc, 128), ts(cc, 128)])
            ps = psum.tile([128, 128], F32, tag="tr_ps")
            nc.tensor.transpose(ps[:], t[:], ident[:])
            nc.vector.tensor_copy(dst_tiles[cc][:, ts(rc, 128)], ps[:])


def build_kernel():
    nc = bacc.Bacc()
    x_ext = nc.declare_dram_parameter("x_c", [S, HID], F32, isOutput=False)
    hn_ext = nc.declare_dram_parameter("hn_c", [1, H], F32, isOutput=False)
    wih_ext = nc.declare_dram_parameter("w_ih", [H3, HID], F32, isOutput=False)
    whh_ext = nc.declare_dram_parameter("w_hh", [H3, H], F32, isOutput=False)
    wex_ext = nc.declare_dram_parameter("w_expr", [H, HID], F32, isOutput=False)
    ema_ext = nc.declare_dram_parameter("ema", [1, E], F32, isOutput=False)

    expr_out = nc.declare_dram_parameter("expr_out", [S, H], F32, isOutput=True)
    hn_out = nc.declare_dram_parameter("hn_out", [1, H], F32, isOutput=True)
    cos_out = nc.declare_dram_parameter("cos_out", [S, E], F32, isOutput=True)
    mult_out = nc.declare_dram_parameter("mult_out", [S, 2], F32, isOutput=True)
    sel_out = nc.declare_dram_parameter("sel_out", [S, 2], I32, isOutput=True)

    xw_d = nc.dram_tensor("xw_d", [S, H3], F32)
    ys_d = nc.dram_tensor("ys_d", [S, H], F32)

    with tile.TileContext(nc) as tc, ExitStack() as ctx:
        const_pool = ctx.enter_context(tc.tile_pool(name="const", bufs=1))
        ident = const_pool.tile([128, 128], F32, tag="ident")
        make_identity(nc, ident[:])
        ident1 = const_pool.tile([1, 1], F32, tag="ident1")
        nc.gpsimd.memset(ident1[:], 1.0)

        work = ctx.enter_context(tc.tile_pool(name="work", bufs=3))
        misc = ctx.enter_context(tc.tile_pool(name="misc", bufs=1))

        # ---- persistent state for the scan ----
        state = ctx.enter_context(tc.tile_pool(name="state", bufs=1))
        h_cols = state.tile([128, 8], F32, tag="h_cols")
        h_hi = state.tile([128, 8], F32R, tag="h_hi")
        h_lo = state.tile([128, 8], F32R, tag="h_lo")
        h_bf = state.tile([128, 8], BF16, tag="h_bf")

        # ---- phase A: x.T, then xW and expression ----
        with tc.tile_pool(name="ph1", bufs=1) as ph1, \
                tc.tile_pool(name="psA", bufs=2, space="PSUM") as psum:
            tp = (work, psum, None)
            xT = [ph1.tile([128, S], F32, tag=f"xT{k}", name=f"xT{k}") for k in range(8)]
            _transpose_to(nc, tc, tp, x_ext, xT, S, HID, ident)

            with tc.tile_pool(name="w1", bufs=1) as w1p:
                w1 = [w1p.tile([128, H3], F32, tag=f"w1_{k}", name=f"w1_{k}") for k in range(8)]
                # w1[k] = w_ih.T chunk: [128 hid, 3072 out]
                for oc in range(H3 // 128):
                    for hc in range(8):
                        t = work.tile([128, 128], F32, tag="tr_ld")
                        nc.sync.dma_start(t[:], wih_ext[ts(oc, 128), ts(hc, 128)])
                        ps = psum.tile([128, 128], F32, tag="tr_ps")
                        nc.tensor.transpose(ps[:], t[:], ident[:])
                        nc.vector.tensor_copy(w1[hc][:, ts(oc, 128)], ps[:])
                # xW = x @ w_ih.T : [S, 3072]
                for tck in range(8):
                    for n in range(6):
                        ps = psum.tile([128, 512], F32, tag="mm_ps")
                        for hc in range(8):
                            nc.tensor.matmul(ps[:], xT[hc][:, ts(tck, 128)],
                                             w1[hc][:, ts(n, 512)],
                                             start=(hc == 0), stop=(hc == 7))
                        xw_t = work.tile([128, 512], F32, tag="xw_t")
                        nc.vector.tensor_copy(xw_t[:], ps[:])
                        nc.sync.dma_start(xw_d[ts(tck, 128), ts(n, 512)], xw_t[:])

            with tc.tile_pool(name="we", bufs=1) as wep:
                we = [wep.tile([128, H], F32, tag=f"we_{k}", name=f"we_{k}") for k in range(8)]
                _transpose_to(nc, tc, tp, wex_ext, we, H, HID, ident)
                for tck in range(8):
                    for n in range(2):
                        ps = psum.tile([128, 512], F32, tag="mm_ps")
                        for hc in range(8):
                            nc.tensor.matmul(ps[:], xT[hc][:, ts(tck, 128)],
                                             we[hc][:, ts(n, 512)],
                                             start=(hc == 0), stop=(hc == 7))
                        ex_t = work.tile([128, 512], F32, tag="ex_t")
                        nc.vector.tensor_copy(ex_t[:], ps[:])
                        nc.sync.dma_start(expr_out[ts(tck, 128), ts(n, 512)], ex_t[:])

        # ---- phase B: W2 split build (freed after the scan) ----
        ctx2 = ExitStack()
        w2_pool = ctx2.enter_context(tc.tile_pool(name="w2", bufs=1))
        w2hi = [w2_pool.tile([128, H3], F32R, tag=f"w2hi{k}", name=f"w2hi{k}") for k in range(8)]
        w2lo = [w2_pool.tile([128, H3], BF16, tag=f"w2lo{k}", name=f"w2lo{k}") for k in range(8)]
        with tc.tile_pool(name="w2tmp", bufs=1) as w2t, \
                tc.tile_pool(name="psB", bufs=2, space="PSUM") as psum:
            for hc in range(8):
                w2f = w2t.tile([128, H3], F32, tag="w2f")
                for oc in range(H3 // 128):
                    t = work.tile([128, 128], F32, tag="tr_ld")
                    nc.sync.dma_start(t[:], whh_ext[ts(oc, 128), ts(hc, 128)])
                    ps = psum.tile([128, 128], F32, tag="tr_ps")
                    nc.tensor.transpose(ps[:], t[:], ident[:])
                    nc.vector.tensor_copy(w2f[:, ts(oc, 128)], ps[:])
                nc.vector.tensor_copy(w2hi[hc][:], w2f[:])
                nc.vector.tensor_tensor(w2lo[hc][:], w2f[:],
                                        w2hi[hc][:].bitcast(F32), ALU.subtract)

        # ---- initial h ----
        loop_pool = ctx2.enter_context(tc.tile_pool(name="loop", bufs=1))
        pg_pool = ctx2.enter_context(tc.tile_pool(name="pg", bufs=2, space="PSUM"))
        tp_pool = ctx2.enter_context(tc.tile_pool(name="tp", bufs=1, space="PSUM"))
        h_row0 = misc.tile([1, H], F32, tag="h_row0")
        nc.sync.dma_start(h_row0[:], hn_ext[:])
        ps_h0 = tp_pool.tile([128, 8], F32, tag="tp_n")
        for k in range(8):
            nc.tensor.transpose(ps_h0[:, k:k + 1], h_row0[0:1, ts(k, 128)], ident1[:])
        nc.vector.tensor_copy(h_cols[:], ps_h0[:])
        nc.vector.tensor_copy(h_hi[:], h_cols[:])
        nc.vector.tensor_tensor(h_lo[:], h_cols[:], h_hi[:].bitcast(F32), ALU.subtract)
        nc.vector.tensor_copy(h_bf[:], h_cols[:])

        # ---- phase C: the scan ----
        def scan_body(i):
            xw_row = loop_pool.tile([1, H3], F32, tag="xw_row")
            nc.sync.dma_start(xw_row[:], xw_d[ds(i, 1), :])

            # hW preacts per gate: r (cols 0:1024), z (1024:2048), n (2048:3072)
            pg = {}
            for gname, gofs in (("r", 0), ("n", 2048), ("z", 1024)):
                ps = pg_pool.tile([1, 1024], F32, tag="pg", name="pg_t")
                for half in range(2):
                    sl = ts(half, 512)
                    for hc in range(8):
                        nc.tensor.matmul(
                            ps[0:1, sl], h_hi[:, hc:hc + 1],
                            w2hi[hc][:, ds(gofs + half * 512, 512)],
                            start=(hc == 0), stop=False, skip_group_check=True)
                    for hc in range(8):
                        nc.tensor.matmul(
                            ps[0:1, sl], h_lo[:, hc:hc + 1],
                            w2hi[hc][:, ds(gofs + half * 512, 512)],
                            start=False, stop=False, skip_group_check=True)
                    for hc in range(8):
                        nc.tensor.matmul(
                            ps[0:1, sl], h_bf[:, hc:hc + 1],
                            w2lo[hc][:, ds(gofs + half * 512, 512)],
                            start=False, stop=(hc == 7), skip_group_check=True)
                pg[gname] = ps

            # r gate entirely in flat [1,1024] layout
            pre_r = loop_pool.tile([1, H], F32, tag="pre_r")
            nc.vector.tensor_tensor(pre_r[:], pg["r"][:], xw_row[0:1, 0:1024], ALU.add)
            r_flat = loop_pool.tile([1, H], F32, tag="r_flat")
            nc.scalar.activation(r_flat[:], pre_r[:], AF.Sigmoid)

            # n: tmp = xn + r * hn_  (flat), then transpose, tanh in T-land
            n_tmp = loop_pool.tile([1, H], F32, tag="n_tmp")
            nc.vector.tensor_tensor(n_tmp[:], r_flat[:], pg["n"][:], ALU.mult)
            nc.vector.tensor_tensor(n_tmp[:], n_tmp[:], xw_row[0:1, 2048:3072], ALU.add)
            ps_n = tp_pool.tile([128, 8], F32, tag="tp_n")
            for k in range(8):
                nc.tensor.transpose(ps_n[:, k:k + 1], n_tmp[0:1, ts(k, 128)], ident1[:])
            n_T = loop_pool.tile([128, 8], F32, tag="n_T")
            nc.scalar.activation(n_T[:], ps_n[:], AF.Tanh)

            # z: preact flat, transpose, sigmoid in T-land
            pre_z = loop_pool.tile([1, H], F32, tag="pre_z")
            nc.vector.tensor_tensor(pre_z[:], pg["z"][:], xw_row[0:1, 1024:2048], ALU.add)
            ps_z = tp_pool.tile([128, 8], F32, tag="tp_z")
            for k in range(8):
                nc.tensor.transpose(ps_z[:, k:k + 1], pre_z[0:1, ts(k, 128)], ident1[:])
            z_T = loop_pool.tile([128, 8], F32, tag="z_T")
            nc.scalar.activation(z_T[:], ps_z[:], AF.Sigmoid)

            # h' = n + z*(h - n)
            hmn = loop_pool.tile([128, 8], F32, tag="hmn")
            nc.vector.tensor_tensor(hmn[:], h_cols[:], n_T[:], ALU.subtract)
            nc.vector.tensor_tensor(hmn[:], z_T[:], hmn[:], ALU.mult)
            nc.vector.tensor_tensor(h_cols[:], n_T[:], hmn[:], ALU.add)
            nc.vector.tensor_copy(h_hi[:], h_cols[:])
            nc.vector.tensor_tensor(h_lo[:], h_cols[:], h_hi[:].bitcast(F32),
                                    ALU.subtract)
            nc.vector.tensor_copy(h_bf[:], h_cols[:])

            # ys[i] = h  (transpose back to a flat row)
            ps_y = tp_pool.tile([8, 128], F32, tag="tp_y")
            nc.tensor.transpose(ps_y[:], h_cols[:], ident[:])
            y_row = loop_pool.tile([8, 128], F32, tag="y_row")
            nc.vector.tensor_copy(y_row[:], ps_y[:])
            nc.sync.dma_start(
                ys_d[ds(i, 1), :].rearrange("a (p f) -> (a p) f", p=8), y_row[:])

        with tc.For_i(0, S, 1) as i:
            scan_body(i)

        nc.sync.dma_start(hn_out[:], ys_d[S - 1:S, :])
        ctx2.close()

        # ---- phase D: per-token scores ----
        dn_pool = ctx.enter_context(tc.tile_pool(name="dn", bufs=2))
        iit = const_pool.tile([128, E], F32, tag="iit")
        for e in range(E):
            nc.gpsimd.memset(iit[:, e:e + 1], float(e))

        adj = const_pool.tile([1, E], F32, tag="adj")
        ema_t = const_pool.tile([1, E], F32, tag="ema_t")
        nc.sync.dma_start(ema_t[:], ema_ext[:])
        ema_sum = const_pool.tile([1, 1], F32, tag="ema_sum")
        nc.vector.tensor_reduce(ema_sum[:], ema_t[:], AX.X, ALU.add)
        ema_inv = const_pool.tile([1, 1], F32, tag="ema_inv")
        nc.vector.reciprocal(ema_inv[:], ema_sum[:])
        # adj = ema / total * (0.01 * E)
        nc.vector.tensor_scalar(adj[:], ema_t[:], ema_inv[:], 0.08,
                                ALU.mult, ALU.mult)
        ones_row = const_pool.tile([1, 128], F32, tag="ones_row")
        nc.gpsimd.memset(ones_row[:], 1.0)
        adjb = const_pool.tile([128, E], F32, tag="adjb")
        with tc.tile_pool(name="psD", bufs=1, space="PSUM") as psD:
            ps_a = psD.tile([128, E], F32, tag="ps_a")
            nc.tensor.matmul(ps_a[:], ones_row[:], adj[:], start=True, stop=True)
            nc.vector.tensor_copy(adjb[:], ps_a[:])

        for tck in range(8):
            Y = dn_pool.tile([128, H], F32, tag="Y")
            nc.sync.dma_start(Y[:], ys_d[ts(tck, 128), :])
            Ex = dn_pool.tile([128, H], F32, tag="Ex")
            nc.sync.dma_start(Ex[:], expr_out[ts(tck, 128), :])

            prod = dn_pool.tile([128, H], F32, tag="prod")
            ny2 = dn_pool.tile([128, E], F32, tag="ny2")
            ne2 = dn_pool.tile([128, E], F32, tag="ne2")
            dt_ = dn_pool.tile([128, E], F32, tag="dt_")
            nc.vector.tensor_tensor(prod[:], Y[:], Y[:], ALU.mult)
            for e in range(E):
                nc.vector.tensor_reduce(ny2[:, e:e + 1], prod[:, ts(e, 128)],
                                        AX.X, ALU.add)
            nc.vector.tensor_tensor(prod[:], Ex[:], Ex[:], ALU.mult)
            for e in range(E):
                nc.vector.tensor_reduce(ne2[:, e:e + 1], prod[:, ts(e, 128)],
                                        AX.X, ALU.add)
            nc.vector.tensor_tensor(prod[:], Ex[:], Y[:], ALU.mult)
            for e in range(E):
                nc.vector.tensor_reduce(dt_[:, e:e + 1], prod[:, ts(e, 128)],
                                        AX.X, ALU.add)

            ny = dn_pool.tile([128, E], F32, tag="ny")
            nc.scalar.activation(ny[:], ny2[:], AF.Sqrt)
            nc.vector.tensor_scalar_max(ny[:], ny[:], 1e-12)
            ne = dn_pool.tile([128, E], F32, tag="ne")
            nc.scalar.activation(ne[:], ne2[:], AF.Sqrt)
            nc.vector.tensor_scalar_max(ne[:], ne[:], 1e-8)
            den = dn_pool.tile([128, E], F32, tag="den")
            nc.vector.tensor_tensor(den[:], ny[:], ne[:], ALU.mult)
            dinv = dn_pool.tile([128, E], F32, tag="dinv")
            nc.vector.reciprocal(dinv[:], den[:])
            cs = dn_pool.tile([128, E], F32, tag="cs")
            nc.vector.tensor_tensor(cs[:], dt_[:], dinv[:], ALU.mult)
            nc.vector.tensor_scalar(cs[:], cs[:], -1.0, 1.0, ALU.mult, ALU.add)
            nc.sync.dma_start(cos_out[ts(tck, 128), :], cs[:])

            # domain scores and top-2
            sc = dn_pool.tile([128, E], F32, tag="sc")
            nc.vector.tensor_scalar_mul(sc[:], cs[:], 9.0)

            def top1(s_t, tag):
                v = dn_pool.tile([128, 1], F32, tag=tag + "v")
                nc.vector.tensor_reduce(v[:], s_t[:], AX.X, ALU.max)
                eq = dn_pool.tile([128, E], F32, tag=tag + "eq")
                nc.vector.tensor_scalar(eq[:], s_t[:], v[:], None, ALU.is_equal)
                cand = dn_pool.tile([128, E], F32, tag=tag + "cand")
                nc.vector.tensor_scalar(cand[:], eq[:], -100.0, 100.0,
                                        ALU.mult, ALU.add)
                nc.vector.tensor_tensor(cand[:], cand[:], iit[:], ALU.add)
                idx = dn_pool.tile([128, 1], F32, tag=tag + "idx")
                nc.vector.tensor_reduce(idx[:], cand[:], AX.X, ALU.min)
                m = dn_pool.tile([128, E], F32, tag=tag + "m")
                nc.vector.tensor_scalar(m[:], iit[:], idx[:], None, ALU.is_equal)
                return v, idx, m

            v1, idx1, m1 = top1(sc, "t1")
            sc2 = dn_pool.tile([128, E], F32, tag="sc2")
            big = dn_pool.tile([128, E], F32, tag="big")
            nc.vector.tensor_scalar_mul(big[:], m1[:], 1e30)
            nc.vector.tensor_tensor(sc2[:], sc[:], big[:], ALU.subtract)
            v2, idx2, m2 = top1(sc2, "t2")

            dv = dn_pool.tile([128, 1], F32, tag="dv")
            nc.vector.tensor_tensor(dv[:], v2[:], v1[:], ALU.subtract)
            dexp = dn_pool.tile([128, 1], F32, tag="dexp")
            nc.scalar.activation(dexp[:], dv[:], AF.Exp)
            p1 = dn_pool.tile([128, 1], F32, tag="p1")
            nc.vector.tensor_scalar_add(p1[:], dexp[:], 1.0)
            nc.vector.reciprocal(p1[:], p1[:])
            p2 = dn_pool.tile([128, 1], F32, tag="p2")
            nc.vector.tensor_tensor(p2[:], dexp[:], p1[:], ALU.mult)

            ga = dn_pool.tile([128, E], F32, tag="ga")
            a1 = dn_pool.tile([128, 1], F32, tag="a1")
            nc.vector.tensor_tensor(ga[:], m1[:], adjb[:], ALU.mult)
            nc.vector.tensor_reduce(a1[:], ga[:], AX.X, ALU.add)
            a2 = dn_pool.tile([128, 1], F32, tag="a2")
            nc.vector.tensor_tensor(ga[:], m2[:], adjb[:], ALU.mult)
            nc.vector.tensor_reduce(a2[:], ga[:], AX.X, ALU.add)

            mu = dn_pool.tile([128, 2], F32, tag="mu")
            nc.vector.tensor_tensor(mu[:, 0:1], p1[:], a1[:], ALU.subtract)
            nc.vector.tensor_tensor(mu[:, 1:2], p2[:], a2[:], ALU.subtract)
            nc.sync.dma_start(mult_out[ts(tck, 128), :], mu[:])

            si = dn_pool.tile([128, 2], I32, tag="si")
            nc.vector.tensor_copy(si[:, 0:1], idx1[:])
            nc.vector.tensor_copy(si[:, 1:2], idx2[:])
            nc.sync.dma_start(sel_out[ts(tck, 128), :], si[:])

    nc.compile()
    return nc


def kernel(x, hn, w_ih, w_hh, w_expr, expert_load_ema, top_k):
    assert int(top_k) == 2
    x = np.ascontiguousarray(np.asarray(x, np.float32))
    hn = np.ascontiguousarray(np.asarray(hn, np.float32))
    w_ih = np.ascontiguousarray(np.asarray(w_ih, np.float32))
    w_hh = np.ascontiguousarray(np.asarray(w_hh, np.float32))
    w_expr = np.ascontiguousarray(np.asarray(w_expr, np.float32))
    ema = np.ascontiguousarray(np.asarray(expert_load_ema, np.float32).reshape(1, E))

    if "nc" not in _NC_CACHE:
        _NC_CACHE["nc"] = build_kernel()
    nc = _NC_CACHE["nc"]

    in_maps = []
    for c in range(NCORES):
        in_maps.append({
            "x_c": np.ascontiguousarray(x[c]),
            "hn_c": np.ascontiguousarray(hn[0, c:c + 1, :]),
            "w_ih": w_ih, "w_hh": w_hh, "w_expr": w_expr, "ema": ema,
        })
    res = run_bass_kernel_spmd(nc, in_maps, core_ids=list(range(NCORES)))
    rs = res.results

    B = NCORES
    multiplier = np.concatenate([r["mult_out"] for r in rs], 0)
    selected = np.concatenate([r["sel_out"] for r in rs], 0).astype(np.int32)
    expression = np.concatenate([r["expr_out"] for r in rs], 0).reshape(B * S, E, D)
    hn_o = np.stack([r["hn_out"][0] for r in rs], 0)[None]
    cos_o = np.stack([r["cos_out"] for r in rs], 0)

    penalty = np.float32(8.0)
    v = w_expr.reshape(E, D * HID)
    nrm = np.maximum(np.linalg.norm(v, axis=-1, keepdims=True), 1e-12)
    vn = (v / nrm).astype(np.float32)
    G = vn @ vn.T
    expr_loss = np.float32(np.mean((G - np.eye(E, dtype=np.float32)) ** 2))

    return (multiplier, selected, expression, hn_o, penalty, cos_o, expr_loss)
